# revision 6
# baseline (speedup 1.0000x reference)
"""Two-layer GCN on 8 Trainium2 NeuronCores.

Math: with dinv = rsqrt(1+indeg) and D = diag(dinv),
    layer1:  h1 = relu(D (A+I) D (x @ W1) + b1)
    layer2:  out = log_softmax(D (A+I) D (h1 @ W2) + b2)
Both aggregations are unweighted gather+sums of pre-scaled rows:
    ht = (D x) @ W1            (per-row dinv folded into x on host)
    u  = relu(D^2 m * agg1 + D m b1)        (= D h1, m = valid mask)
    z  = (D agg2) @ W2 + b2                 (aggregate-then-transform)
so the layer-2 table stays H=64 wide and the W2 matmul runs once per
128-node tile after aggregation.

Distribution: degree-sorted round-robin node sharding (core = rank % 8).
Tiles of 128 nodes are batched into groups (<=7 tiles, <=132 gather
columns); each group's gather is ONE batched indirect DMA (SWDGE fixed
cost ~1us amortized over ~15K descriptors of 128B bf16 rows).  The
reduction is a contiguous bf16 pairwise tree (DVE 2x mode) instead of a
strided reduce.  One AllGather exchanges the bf16 u-table between
layers.  Tables use permuted row layouts so every phase-1/2 write DMA
is a single contiguous run.
"""

import os
import sys
import types

sys.path.insert(0, "/opt/trn_rl_repo")

import numpy as np

N = 100000
E = 1600000
D_IN, H, D_OUT = 128, 64, 40
N_CORES = 8
P = 128
SHARD = 12500                 # nodes per core
TILES = 98                    # ceil(12500/128)
SHARD_PAD = TILES * P         # 12544 table rows per core shard
ROWS = N_CORES * SHARD_PAD    # 100352 total table rows
TCHUNK = 2048                 # phase-1 transform chunk (16 subtiles of 128)
NSUB = TCHUNK // P            # 16
NCHUNK = ROWS // TCHUNK       # 49
GCAP = 132                    # max gather index columns per group
GSMAX = 7                     # max tiles per group


def _build_bass(groups, goff, gcols):
    """Build the SPMD Bass program. groups = [(t0, S, Kg)]."""
    import concourse.bass as bass
    import concourse.bacc as bacc
    import concourse.tile as tile
    import concourse.mybir as mybir
    from concourse.masks import make_identity

    f32 = mybir.dt.float32
    bf16 = mybir.dt.bfloat16
    i32 = mybir.dt.int32

    nc = bacc.Bacc("TRN2", target_bir_lowering=False, debug=False,
                   num_devices=N_CORES)

    # ---- kernel I/O ----
    xTs = nc.dram_tensor("xTs", [P, ROWS], bf16, kind="ExternalInput")
    w1 = nc.dram_tensor("w1", [D_IN, H], bf16, kind="ExternalInput")
    w2blk = nc.dram_tensor("w2blk", [2 * H, 2 * D_OUT], bf16, kind="ExternalInput")
    gidx1 = nc.dram_tensor("gidx1", [P, gcols], i32, kind="ExternalInput")
    gidx2 = nc.dram_tensor("gidx2", [P, gcols], i32, kind="ExternalInput")
    a1t = nc.dram_tensor("a1t", [P, TILES], f32, kind="ExternalInput")
    dsl2t = nc.dram_tensor("dsl2t", [P, TILES], f32, kind="ExternalInput")
    B1t = nc.dram_tensor("B1t", [P, TILES * H], f32, kind="ExternalInput")
    b2rep = nc.dram_tensor("b2rep", [P, 6 * 2 * D_OUT], f32, kind="ExternalInput")
    out_ext = nc.dram_tensor("out", [SHARD_PAD, D_OUT], f32, kind="ExternalOutput")

    NPAIR = TILES // 2          # 49 pairs of tiles for the W2 matmul
    ZBLK = 6                    # pairs per PSUM z-block

    with tile.TileContext(nc) as tc:
        with (
            tc.tile_pool(name="const", bufs=1) as cp,
            tc.tile_pool(name="xin", bufs=3) as xp,
            tc.tile_pool(name="stage", bufs=3) as stp,
            tc.tile_pool(name="gat", bufs=2) as gp,
            tc.tile_pool(name="work", bufs=3) as wp,
            tc.tile_pool(name="psum", bufs=2, space="PSUM") as pp,
            tc.tile_pool(name="psumT", bufs=2, space="PSUM") as pt,
            tc.tile_pool(name="psumZ", bufs=2, space="PSUM") as pz,
            tc.tile_pool(name="dram", bufs=1, space="DRAM") as dram,
        ):
            ht = dram.tile([ROWS, H], bf16)          # layer-1 table (replicated)
            ut_own = dram.tile([SHARD_PAD, H], bf16)  # own shard of layer-2 table
            ut = dram.tile([ROWS, H], bf16)           # gathered layer-2 table

            # ---- constants ----
            w1sb = cp.tile([D_IN, H], bf16)
            nc.sync.dma_start(out=w1sb[:], in_=w1[:, :])
            w2sb = cp.tile([2 * H, 2 * D_OUT], bf16)
            nc.sync.dma_start(out=w2sb[:], in_=w2blk[:, :])
            gix1 = cp.tile([P, gcols], i32)
            nc.sync.dma_start(out=gix1[:], in_=gidx1[:, :])
            gix2 = cp.tile([P, gcols], i32)
            nc.sync.dma_start(out=gix2[:], in_=gidx2[:, :])
            a1 = cp.tile([P, TILES], f32)
            nc.sync.dma_start(out=a1[:], in_=a1t[:, :])
            dsl2 = cp.tile([P, TILES], f32)
            nc.sync.dma_start(out=dsl2[:], in_=dsl2t[:, :])
            B1 = cp.tile([P, TILES * H], f32)
            nc.sync.dma_start(out=B1[:], in_=B1t[:, :])
            b2r = cp.tile([P, 6 * 2 * D_OUT], f32)
            nc.sync.dma_start(out=b2r[:], in_=b2rep[:, :])
            ident = cp.tile([P, P], f32)
            make_identity(nc, ident[:])
            tstage = cp.tile([P, TILES * H], f32)     # phase-4 scaled aggregates
            zstage = cp.tile([P, TILES * D_OUT], f32)  # logits
            exstage = cp.tile([P, TILES * D_OUT], f32)  # exp(z - max)

            # ---- phase 1: ht = (D x) @ W1, all rows (replicated) ----
            # ht row for xTs column (c, g, p) is c*TCHUNK + p*NSUB + g so the
            # write lands as one contiguous run per chunk.
            for c in range(NCHUNK):
                xt = xp.tile([P, TCHUNK], bf16, name=f"xt{c}", tag="xt")
                nc.sync.dma_start(out=xt[:], in_=xTs[:, c * TCHUNK:(c + 1) * TCHUNK])
                ps = pp.tile([P, NSUB * H], f32, name=f"ps{c}", tag="ps")
                for g in range(NSUB):
                    nc.tensor.matmul(
                        out=ps[:, g * H:(g + 1) * H],
                        lhsT=xt[:, g * P:(g + 1) * P],
                        rhs=w1sb[:],
                        start=True, stop=True,
                    )
                st = stp.tile([P, NSUB * H], bf16, name=f"st{c}", tag="st")
                nc.scalar.copy(st[:], ps[:])
                nc.sync.dma_start(
                    out=ht[c * TCHUNK:(c + 1) * TCHUNK, :]
                        .rearrange("(p g) f -> p g f", p=P),
                    in_=st[:].rearrange("p (g f) -> p g f", f=H),
                )

            # ---- phase 2: layer-1 aggregate + scale -> u table ----
            def tree_reduce(gb, S, Kg, out3):
                """Sum the Kg 64-elem blocks per tile via contiguous pairwise
                adds in bf16; final add writes f32 into out3 [P, S, 64]."""
                B3 = gb[:, :S * Kg * H].rearrange("p (s q) -> p s q", q=Kg * H)
                k = Kg
                while k > 2:
                    m = (k + 1) // 2
                    h = k - m
                    nc.vector.tensor_add(
                        out=B3[:, :, :h * H],
                        in0=B3[:, :, :h * H],
                        in1=B3[:, :, m * H:(m + h) * H],
                    )
                    k = m
                if k == 2:
                    nc.vector.tensor_add(
                        out=out3, in0=B3[:, :, 0:H], in1=B3[:, :, H:2 * H])
                else:
                    nc.vector.tensor_copy(out=out3, in_=B3[:, :, 0:H])

            for gi, (t0, S, Kg) in enumerate(groups):
                cols = S * Kg
                gb = gp.tile([P, GCAP * H], bf16, name=f"g1_{gi}", tag="gb")
                for c in range(cols):
                    nc.gpsimd.indirect_dma_start(
                        out=gb[:, c * H:(c + 1) * H],
                        out_offset=None,
                        in_=ht[:, :],
                        in_offset=bass.IndirectOffsetOnAxis(
                            ap=gix1[:, goff[gi] + c:goff[gi] + c + 1], axis=0),
                    )
                t2 = wp.tile([P, GSMAX * H], f32, name=f"t2_{gi}", tag="t2")
                t2v = t2[:, :S * H].rearrange("p (s f) -> p s f", f=H)
                tree_reduce(gb, S, Kg, t2v)
                # u = relu(a1*agg + B1)   (a1 = dinv^2*mask, B1 = dinv*mask*b1)
                nc.vector.tensor_tensor(
                    out=t2v, in0=t2v,
                    in1=a1[:, t0:t0 + S].to_broadcast([P, S, H]),
                    op=mybir.AluOpType.mult,
                )
                nc.vector.tensor_add(
                    out=t2v, in0=t2v,
                    in1=B1[:, t0 * H:(t0 + S) * H].rearrange("p (s f) -> p s f", f=H),
                )
                u = wp.tile([P, GSMAX * H], bf16, name=f"u_{gi}", tag="u")
                nc.scalar.activation(u[:, :S * H], t2[:, :S * H],
                                     mybir.ActivationFunctionType.Relu)
                # ut row for (t0+s, p) is t0*128 + p*S + s: contiguous run.
                nc.sync.dma_start(
                    out=ut_own[t0 * P:(t0 + S) * P, :]
                        .rearrange("(p s) f -> p s f", p=P),
                    in_=u[:, :S * H].rearrange("p (s f) -> p s f", f=H),
                )

            # ---- phase 3: exchange layer-2 table ----
            nc.gpsimd.collective_compute(
                "AllGather",
                mybir.AluOpType.bypass,
                replica_groups=[list(range(N_CORES))],
                ins=[ut_own.opt()],
                outs=[ut.opt()],
            )

            # ---- phase 4: layer-2 aggregate + transform + log_softmax ----
            for gi, (t0, S, Kg) in enumerate(groups):
                cols = S * Kg
                gb = gp.tile([P, GCAP * H], bf16, name=f"g2_{gi}", tag="gb")
                for c in range(cols):
                    nc.gpsimd.indirect_dma_start(
                        out=gb[:, c * H:(c + 1) * H],
                        out_offset=None,
                        in_=ut[:, :],
                        in_offset=bass.IndirectOffsetOnAxis(
                            ap=gix2[:, goff[gi] + c:goff[gi] + c + 1], axis=0),
                    )
                tsv = tstage[:, t0 * H:(t0 + S) * H].rearrange(
                    "p (s f) -> p s f", f=H)
                tree_reduce(gb, S, Kg, tsv)
                nc.vector.tensor_tensor(
                    out=tsv, in0=tsv,
                    in1=dsl2[:, t0:t0 + S].to_broadcast([P, S, H]),
                    op=mybir.AluOpType.mult,
                )

            # z = (D agg) @ W2 + b2, two tiles per matmul via block-diag W2
            for q in range((NPAIR + ZBLK - 1) // ZBLK):
                pr0 = q * ZBLK
                npair = min(ZBLK, NPAIR - pr0)
                zp = pz.tile([P, ZBLK * 2 * D_OUT], f32, name=f"zp{q}", tag="zp")
                for j in range(npair):
                    pr = pr0 + j
                    hT = pt.tile([P, P], f32, name=f"hT{pr}", tag="hT")
                    nc.tensor.transpose(
                        out=hT[:], in_=tstage[:, pr * P:(pr + 1) * P],
                        identity=ident[:])
                    lh = wp.tile([P, P], bf16, name=f"lh{pr}", tag="lh")
                    nc.scalar.copy(lh[:], hT[:])
                    nc.tensor.matmul(
                        out=zp[:, j * 2 * D_OUT:(j + 1) * 2 * D_OUT],
                        lhsT=lh[:], rhs=w2sb[:], start=True, stop=True)
                nc.vector.tensor_add(
                    out=zstage[:, pr0 * 2 * D_OUT:(pr0 + npair) * 2 * D_OUT],
                    in0=zp[:, :npair * 2 * D_OUT],
                    in1=b2r[:, :npair * 2 * D_OUT],
                )

            # log_softmax over the 40 columns of each tile, all tiles at once
            z3 = zstage[:].rearrange("p (t f) -> p t f", f=D_OUT)
            nm = cp.tile([P, TILES], f32)
            nc.vector.reduce_max(out=nm[:], in_=z3, axis=mybir.AxisListType.X,
                                 negate=True)
            nc.vector.tensor_add(out=z3, in0=z3,
                                 in1=nm[:].to_broadcast([P, TILES, D_OUT]))
            nc.scalar.activation(exstage[:], zstage[:],
                                 mybir.ActivationFunctionType.Exp)
            ssum = cp.tile([P, TILES], f32)
            nc.vector.reduce_sum(
                out=ssum[:],
                in_=exstage[:].rearrange("p (t f) -> p t f", f=D_OUT),
                axis=mybir.AxisListType.X)
            lse = cp.tile([P, TILES], f32)
            nc.scalar.activation(lse[:], ssum[:],
                                 mybir.ActivationFunctionType.Ln)
            nc.vector.tensor_tensor(
                out=z3, in0=z3,
                in1=lse[:].to_broadcast([P, TILES, D_OUT]),
                op=mybir.AluOpType.subtract,
            )
            # out row for (t, p) is p*98 + t: one contiguous run.
            nc.sync.dma_start(
                out=out_ext[:, :].rearrange("(p t) f -> p t f", p=P),
                in_=z3,
            )

    nc.compile()
    return nc


def _prep(x, edge_index, W1, b1, W2, b2):
    """Host-side sharding/layout prep (index manipulation only)."""
    import ml_dtypes

    src = edge_index[0].astype(np.int64)
    dst = edge_index[1].astype(np.int64)
    indeg = np.bincount(dst, minlength=N)
    dinv = (1.0 / np.sqrt(1.0 + indeg)).astype(np.float32)

    # degree-sorted round-robin shard assignment
    order = np.argsort(-indeg, kind="stable")      # rank -> node
    node_core = np.empty(N, np.int64)
    node_slot = np.empty(N, np.int64)
    node_core[order] = np.arange(N) % N_CORES
    node_slot[order] = np.arange(N) // N_CORES
    xcol = node_core * SHARD_PAD + node_slot        # node -> xTs column

    # ht row permutation: xTs col (c, g, p) -> row c*TCHUNK + p*NSUB + g
    cols = np.arange(ROWS)
    cc, rr = cols // TCHUNK, cols % TCHUNK
    gg, pp = rr // P, rr % P
    pi = cc * TCHUNK + pp * NSUB + gg               # col -> ht row
    ht_row = pi[xcol]                               # node -> ht row

    # tile schedule: K per tile = max (deg+1) over the tile across all cores
    rank_deg = indeg[order]
    tile_k = np.array([int(rank_deg[t * P * N_CORES]) + 1 for t in range(TILES)])

    # group tiles: S <= GSMAX, S*Kg <= GCAP (Kg = K of first tile, sorted desc)
    groups = []
    t = 0
    while t < TILES:
        Kg = int(tile_k[t])
        S = 1
        while (t + S < TILES and S < GSMAX and (S + 1) * Kg <= GCAP
               and Kg - tile_k[min(t + S, TILES - 1)] <= 3):
            S += 1
        groups.append((t, S, Kg))
        t += S
    goff = np.zeros(len(groups), np.int64)
    off = 0
    for i, (t0, S, Kg) in enumerate(groups):
        goff[i] = off
        off += S * Kg
    gcols = int(off)

    # per-tile lookup tables
    T0 = np.zeros(TILES, np.int64)   # group start tile
    KG = np.zeros(TILES, np.int64)   # group Kg
    GOF = np.zeros(TILES, np.int64)  # group column offset
    GS = np.zeros(TILES, np.int64)   # group size
    for i, (t0, S, Kg) in enumerate(groups):
        T0[t0:t0 + S] = t0
        KG[t0:t0 + S] = Kg
        GOF[t0:t0 + S] = goff[i]
        GS[t0:t0 + S] = S

    # ut row: (core, tile t, part p) -> core*SHARD_PAD + t0*128 + p*S + (t-t0)
    t_of = node_slot // P
    p_of = node_slot % P
    ut_row = (node_core * SHARD_PAD + T0[t_of] * P + p_of * GS[t_of]
              + (t_of - T0[t_of]))

    # zero rows (rows that are guaranteed all-zero in each table)
    zero_ht = int(pi[0 * SHARD_PAD + SHARD])        # core-0 junk column
    used = np.zeros(SHARD_PAD, bool)
    own0 = node_core == 0
    used[ut_row[own0] - 0 * SHARD_PAD] = True
    zero_ut = int(np.flatnonzero(~used)[0])

    gidx1_all = np.full((N_CORES, P, gcols), zero_ht, np.int32)
    gidx2_all = np.full((N_CORES, P, gcols), zero_ut, np.int32)

    # self loops at k = 0 of each tile's column block
    col0 = GOF[t_of] + (t_of - T0[t_of]) * KG[t_of]
    gidx1_all[node_core, p_of, col0] = ht_row.astype(np.int32)
    gidx2_all[node_core, p_of, col0] = ut_row.astype(np.int32)

    # edges bucketed by (core, slot), k = 1 + within-destination order
    e_core = node_core[dst]
    e_slot = node_slot[dst]
    eo = np.lexsort((src, e_slot, e_core))
    sc, ss, ssrc = e_core[eo], e_slot[eo], src[eo]
    grp = sc * SHARD_PAD + ss
    first = np.ones(len(grp), bool)
    first[1:] = grp[1:] != grp[:-1]
    gstart = np.flatnonzero(first)
    within = np.arange(len(grp)) - np.repeat(
        gstart, np.diff(np.append(gstart, len(grp))))
    st_of = ss // P
    sp_of = ss % P
    colk = GOF[st_of] + (st_of - T0[st_of]) * KG[st_of] + 1 + within
    gidx1_all[sc, sp_of, colk] = ht_row[ssrc].astype(np.int32)
    gidx2_all[sc, sp_of, colk] = ut_row[ssrc].astype(np.int32)

    # xTs: dinv-prescaled x in xcol order, zero junk cols, transposed bf16
    X = np.zeros((ROWS, D_IN), np.float32)
    X[xcol] = x * dinv[:, None]
    xTs = np.ascontiguousarray(X.T).astype(ml_dtypes.bfloat16)

    # per-core scale tables (zero at junk slots = mask folded in)
    a1_all = np.zeros((N_CORES, P, TILES), np.float32)
    dsl2_all = np.zeros((N_CORES, P, TILES), np.float32)
    B1_all = np.zeros((N_CORES, P, TILES, H), np.float32)
    a1_all[node_core, p_of, t_of] = dinv * dinv
    dsl2_all[node_core, p_of, t_of] = dinv
    B1_all[node_core, p_of, t_of] = dinv[:, None] * b1[None, :]
    B1_all = B1_all.reshape(N_CORES, P, TILES * H)

    w1bf = W1.astype(ml_dtypes.bfloat16)
    w2blk = np.zeros((2 * H, 2 * D_OUT), np.float32)
    w2blk[:H, :D_OUT] = W2
    w2blk[H:, D_OUT:] = W2
    w2blk = w2blk.astype(ml_dtypes.bfloat16)
    b2rep = np.tile(b2[None, :], (P, 12)).astype(np.float32)

    return dict(
        xTs=xTs, w1bf=w1bf, w2blk=w2blk, b2rep=b2rep,
        gidx1_all=gidx1_all, gidx2_all=gidx2_all,
        a1_all=a1_all, dsl2_all=dsl2_all, B1_all=B1_all,
        groups=groups, goff=goff, gcols=gcols,
        node_core=node_core, t_of=t_of, p_of=p_of,
    )


_CACHE = {}


def kernel(x, edge_index, W1, b1, W2, b2):
    # register the axon NTFF hook shim so bass_utils imports cleanly
    if "antenv.axon_hooks" not in sys.modules:
        m = types.ModuleType("antenv.axon_hooks")
        m._h = None
        m.set_axon_ntff_profile_hook = lambda h: setattr(m, "_h", h)
        m.get_axon_ntff_profile_hook = lambda: m._h
        sys.modules["antenv.axon_hooks"] = m

    from concourse.bass_utils import run_bass_kernel_spmd

    x = np.asarray(x, np.float32)
    edge_index = np.asarray(edge_index, np.int32)
    W1 = np.asarray(W1, np.float32)
    b1 = np.asarray(b1, np.float32)
    W2 = np.asarray(W2, np.float32)
    b2 = np.asarray(b2, np.float32)

    pr = _prep(x, edge_index, W1, b1, W2, b2)

    key = ("gcn2", pr["gcols"], tuple(pr["groups"]))
    if key not in _CACHE:
        _CACHE[key] = _build_bass(pr["groups"], pr["goff"], pr["gcols"])
    nc = _CACHE[key]

    in_maps = []
    for c in range(N_CORES):
        in_maps.append({
            "xTs": pr["xTs"],
            "w1": pr["w1bf"], "w2blk": pr["w2blk"], "b2rep": pr["b2rep"],
            "gidx1": pr["gidx1_all"][c],
            "gidx2": pr["gidx2_all"][c],
            "a1t": pr["a1_all"][c],
            "dsl2t": pr["dsl2_all"][c],
            "B1t": pr["B1_all"][c],
        })
    res = run_bass_kernel_spmd(nc, in_maps, core_ids=list(range(N_CORES)),
                               trace=bool(int(os.environ.get("GCN_TRACE", "0"))))
    kernel.last_exec_ns = res.exec_time_ns

    out = np.empty((N, D_OUT), np.float32)
    nc_arr = pr["node_core"]
    rows = pr["p_of"] * TILES + pr["t_of"]   # out_ext row = p*98 + t
    for c in range(N_CORES):
        own = np.flatnonzero(nc_arr == c)
        out[own] = res.results[c]["out"][rows[own]]
    return out


if __name__ == "__main__":
    rng = np.random.default_rng(0)
    xs = rng.standard_normal((N, D_IN)).astype(np.float32)
    ei = rng.integers(0, N, (2, E)).astype(np.int32)
    w1 = rng.standard_normal((D_IN, H)).astype(np.float32) / np.sqrt(D_IN)
    w2 = rng.standard_normal((H, D_OUT)).astype(np.float32) / np.sqrt(H)
    o = kernel(xs, ei, w1, np.zeros(H, np.float32), w2, np.zeros(D_OUT, np.float32))
    print(o.shape, kernel.last_exec_ns)


# revision 10
# speedup vs baseline: 1.0324x; 1.0324x over previous
"""Two-layer GCN on 8 Trainium2 NeuronCores.

Math refactor: with dinv = rsqrt(1+indeg), the PyG GCNConv is
    conv(h)[n] = dinv[n] * ( sum_{e: dst=n} t[src_e] + t[n] ) + b,
    where t = dinv ⊙ (h @ W)
so aggregation is a pure unweighted gather+sum over (in-edges ∪ self).

Distribution: nodes are degree-sorted and round-robined over the 8 cores
(core = rank % 8) so every core sees an identical tile schedule.  Each
core owns 12500 nodes = 98 tiles of 128.  Table row space is
[core * SHARD_PAD + slot] so an AllGather of per-core shards yields the
full table.  Both layers share one slot-index array.

Per tile (K = max degree+1 in tile): K single-column indirect DMA row
gathers land [128, K, F] in SBUF; a strided DVE reduce sums over K; ACT
ops apply dinv/bias/relu; PE computes h1 @ W2 via transpose+matmul.  A
single AllGather exchanges the second-layer table between layers.
"""

import os
import sys
import types

sys.path.insert(0, "/opt/trn_rl_repo")

import numpy as np

N = 100000
E = 1600000
D_IN, H, D_OUT = 128, 64, 40
N_CORES = 8
P = 128
SHARD = 12500                 # nodes per core
TILES = 98                    # ceil(12500/128); last tile has 84 real nodes
SHARD_PAD = TILES * P         # 12544 table rows per core shard
ROWS = N_CORES * SHARD_PAD    # 100352 total table rows
TCHUNK = 1024                 # transform chunk (8 sub-tiles of 128)


def _build_bass(kcols, tile_off, tile_k):
    """Build the SPMD Bass program. kcols = total gather-index columns."""
    import concourse.bass as bass
    import concourse.bacc as bacc
    import concourse.tile as tile
    import concourse.mybir as mybir
    from concourse.masks import make_identity

    f32 = mybir.dt.float32
    bf16 = mybir.dt.bfloat16
    i32 = mybir.dt.int32

    nc = bacc.Bacc("TRN2", target_bir_lowering=False, debug=False,
                   num_devices=N_CORES)

    # ---- kernel I/O ----
    xT = nc.dram_tensor("xT", [P, ROWS], bf16, kind="ExternalInput")
    W1 = nc.dram_tensor("W1", [D_IN, H], f32, kind="ExternalInput")
    W2 = nc.dram_tensor("W2", [H, D_OUT], f32, kind="ExternalInput")
    b1r = nc.dram_tensor("b1r", [P, H], f32, kind="ExternalInput")
    b2r = nc.dram_tensor("b2r", [P, D_OUT], f32, kind="ExternalInput")
    cntg = nc.dram_tensor("cntg", [P, ROWS // P], i32, kind="ExternalInput")
    cnts = nc.dram_tensor("cnts", [P, TILES], i32, kind="ExternalInput")
    masks = nc.dram_tensor("masks", [P, TILES], f32, kind="ExternalInput")
    gidx = nc.dram_tensor("gidx", [P, kcols], i32, kind="ExternalInput")
    out_ext = nc.dram_tensor("out", [SHARD, D_OUT], f32, kind="ExternalOutput")

    with tile.TileContext(nc) as tc:
        with (
            tc.tile_pool(name="const", bufs=1) as cp,
            tc.tile_pool(name="xin", bufs=3) as xp,
            tc.tile_pool(name="stage", bufs=3) as stp,
            tc.tile_pool(name="gat", bufs=6) as gp,
            tc.tile_pool(name="ep", bufs=3) as ep,
            tc.tile_pool(name="psum", bufs=2, space="PSUM") as pp,
            tc.tile_pool(name="psum2", bufs=2, space="PSUM") as pp2,
            tc.tile_pool(name="dram", bufs=1, space="DRAM") as dram,
        )        :
            ht = dram.tile([ROWS, H], f32)             # layer-1 table (local)
            zt_in = dram.tile([SHARD_PAD, D_OUT], f32)  # layer-2 shard bounce
            zt = dram.tile([ROWS, D_OUT], f32)          # layer-2 table (gathered)

            # ---- constants ----
            w1sb = cp.tile([D_IN, H], f32)
            nc.sync.dma_start(out=w1sb[:], in_=W1[:, :])
            w1bf = cp.tile([D_IN, H], bf16)
            nc.vector.tensor_copy(out=w1bf[:], in_=w1sb[:])
            w2sb = cp.tile([H, D_OUT], f32)
            nc.sync.dma_start(out=w2sb[:], in_=W2[:, :])
            b1sb = cp.tile([P, H], f32)
            nc.sync.dma_start(out=b1sb[:], in_=b1r[:, :])
            b2sb = cp.tile([P, D_OUT], f32)
            nc.sync.dma_start(out=b2sb[:], in_=b2r[:, :])
            ident = cp.tile([P, P], f32)
            make_identity(nc, ident[:])
            gix = cp.tile([P, kcols], i32)
            nc.sync.dma_start(out=gix[:], in_=gidx[:, :])
            msk = cp.tile([P, TILES], f32)
            nc.sync.dma_start(out=msk[:], in_=masks[:, :])

            # dinv tables: global (table-row order) and shard (tile order)
            cg = cp.tile([P, ROWS // P], i32)
            nc.sync.dma_start(out=cg[:], in_=cntg[:, :])
            cgf = cp.tile([P, ROWS // P], f32)
            nc.vector.tensor_copy(out=cgf[:], in_=cg[:])
            nc.scalar.activation(cgf[:], cgf[:], mybir.ActivationFunctionType.Sqrt,
                                 bias=1.0, scale=1.0)
            dg = cp.tile([P, ROWS // P], f32)
            nc.vector.reciprocal(dg[:], cgf[:])

            cs = cp.tile([P, TILES], i32)
            nc.sync.dma_start(out=cs[:], in_=cnts[:, :])
            csf = cp.tile([P, TILES], f32)
            nc.vector.tensor_copy(out=csf[:], in_=cs[:])
            nc.scalar.activation(csf[:], csf[:], mybir.ActivationFunctionType.Sqrt,
                                 bias=1.0, scale=1.0)
            ds = cp.tile([P, TILES], f32)
            nc.vector.reciprocal(ds[:], csf[:])

            # ---- phase 1: ht = dinv ⊙ (x @ W1), all rows (replicated) ----
            nsub = TCHUNK // P
            for c in range(ROWS // TCHUNK):
                xt_sb = xp.tile([P, TCHUNK], bf16, name=f"xt{c}", tag="xt")
                nc.sync.dma_start(out=xt_sb[:], in_=xT[:, c * TCHUNK:(c + 1) * TCHUNK])
                ps = pp.tile([P, TCHUNK // 2], f32, name=f"ps{c}", tag="ps")
                st = stp.tile([P, TCHUNK // 2], f32, name=f"st{c}", tag="st")
                for g in range(nsub):
                    nc.tensor.matmul(
                        out=ps[:, g * H:(g + 1) * H],
                        lhsT=xt_sb[:, g * P:(g + 1) * P],
                        rhs=w1bf[:],
                        start=True, stop=True,
                    )
                # dinv is folded into xT on the host, so one plain wide
                # PSUM->SBUF copy replaces 8 per-subtile scaled copies.
                nc.scalar.copy(st[:], ps[:])
                nc.sync.dma_start(
                    out=ht[c * TCHUNK:(c + 1) * TCHUNK, :]
                        .rearrange("(g p) f -> p g f", p=P),
                    in_=st[:].rearrange("p (g f) -> p g f", f=H),
                )

            # ---- phase 2: layer-1 aggregation + layer-2 transform ----
            for t in range(TILES):
                K = tile_k[t]
                gb = gp.tile([P, K * H], f32, name=f"g1_{t}", tag="g1")
                for k in range(K):
                    nc.gpsimd.indirect_dma_start(
                        out=gb[:, k * H:(k + 1) * H],
                        out_offset=None,
                        in_=ht[:, :],
                        in_offset=bass.IndirectOffsetOnAxis(
                            ap=gix[:, tile_off[t] + k: tile_off[t] + k + 1], axis=0),
                    )
                red = ep.tile([P, H], f32, name=f"r1_{t}", tag="r1")
                nc.vector.reduce_sum(
                    out=red[:],
                    in_=gb[:].rearrange("p (k f) -> p f k", k=K),
                    axis=mybir.AxisListType.X,
                )
                # h1 = relu(red*dinv + b1) * mask
                h1 = ep.tile([P, H], f32, name=f"h1_{t}", tag="h1")
                nc.scalar.activation(h1[:], red[:], mybir.ActivationFunctionType.Copy,
                                     scale=ds[:, t:t + 1])
                nc.vector.tensor_add(out=h1[:], in0=h1[:], in1=b1sb[:])
                nc.scalar.activation(h1[:], h1[:], mybir.ActivationFunctionType.Relu)
                nc.scalar.activation(h1[:], h1[:], mybir.ActivationFunctionType.Copy,
                                     scale=msk[:, t:t + 1])
                # z2 = dinv ⊙ (h1 @ W2): transpose h1 then matmul
                hT_ps = pp2.tile([H, P], f32, name=f"hT_{t}", tag="hT")
                nc.tensor.transpose(out=hT_ps[:], in_=h1[:], identity=ident[:])
                hT = ep.tile([H, P], f32, name=f"hTs_{t}", tag="hTs")
                nc.scalar.copy(hT[:], hT_ps[:])
                z_ps = pp2.tile([P, D_OUT], f32, name=f"z_{t}", tag="z")
                nc.tensor.matmul(out=z_ps[:], lhsT=hT[:], rhs=w2sb[:],
                                 start=True, stop=True)
                zst = ep.tile([P, D_OUT], f32, name=f"zs_{t}", tag="zs")
                nc.scalar.activation(zst[:], z_ps[:],
                                     mybir.ActivationFunctionType.Copy,
                                     scale=ds[:, t:t + 1])
                nc.sync.dma_start(out=zt_in[t * P:(t + 1) * P, :], in_=zst[:])

            # ---- phase 3: exchange layer-2 table ----
            nc.gpsimd.collective_compute(
                "AllGather",
                mybir.AluOpType.bypass,
                replica_groups=[list(range(N_CORES))],
                ins=[zt_in.opt()],
                outs=[zt.opt()],
            )

            # ---- phase 4: layer-2 aggregation + log_softmax ----
            for t in range(TILES):
                K = tile_k[t]
                # self-loop slot (k=0) is this core's own shard rows: affine
                # read from the local pre-AllGather bounce, saving one
                # indirect DMA per tile (SWDGE sem ticks are a scarce 16-bit
                # resource) — remaining K-1 slots are indirect row gathers.
                zself = ep.tile([P, D_OUT], f32, name=f"sf_{t}", tag="sf")
                nc.sync.dma_start(out=zself[:], in_=zt_in[t * P:(t + 1) * P, :])
                red2 = ep.tile([P, D_OUT], f32, name=f"r2_{t}", tag="r2")
                if K > 1:
                    gb2 = gp.tile([P, (K - 1) * D_OUT], f32, name=f"g2_{t}", tag="g2")
                    for k in range(1, K):
                        nc.gpsimd.indirect_dma_start(
                            out=gb2[:, (k - 1) * D_OUT: k * D_OUT],
                            out_offset=None,
                            in_=zt[:, :],
                            in_offset=bass.IndirectOffsetOnAxis(
                                ap=gix[:, tile_off[t] + k: tile_off[t] + k + 1], axis=0),
                        )
                    nc.vector.reduce_sum(
                        out=red2[:],
                        in_=gb2[:].rearrange("p (k f) -> p f k", k=K - 1),
                        axis=mybir.AxisListType.X,
                    )
                    nc.vector.tensor_add(out=red2[:], in0=red2[:], in1=zself[:])
                else:
                    nc.vector.tensor_copy(out=red2[:], in_=zself[:])
                z = ep.tile([P, D_OUT], f32, name=f"zz_{t}", tag="zz")
                nc.scalar.activation(z[:], red2[:], mybir.ActivationFunctionType.Copy,
                                     scale=ds[:, t:t + 1])
                nc.vector.tensor_add(out=z[:], in0=z[:], in1=b2sb[:])
                # log_softmax over the 40 columns
                nm = ep.tile([P, 1], f32, name=f"nm_{t}", tag="nm")
                nc.vector.reduce_max(out=nm[:], in_=z[:], axis=mybir.AxisListType.X,
                                     negate=True)
                ex = ep.tile([P, D_OUT], f32, name=f"ex_{t}", tag="ex")
                ssum = ep.tile([P, 1], f32, name=f"ss_{t}", tag="ss")
                nc.scalar.activation(ex[:], z[:], mybir.ActivationFunctionType.Exp,
                                     bias=nm[:], scale=1.0, accum_out=ssum[:])
                lse = ep.tile([P, 1], f32, name=f"ls_{t}", tag="ls")
                nc.scalar.activation(lse[:], ssum[:], mybir.ActivationFunctionType.Ln)
                o = ep.tile([P, D_OUT], f32, name=f"o_{t}", tag="o")
                nc.vector.tensor_scalar(
                    out=o[:], in0=z[:],
                    scalar1=nm[:, :1], scalar2=lse[:, :1],
                    op0=mybir.AluOpType.add, op1=mybir.AluOpType.subtract,
                )
                rows = min(SHARD - t * P, P)
                nc.sync.dma_start(out=out_ext[t * P: t * P + rows, :],
                                  in_=o[:rows, :])

    nc.compile()
    return nc


def _prep(x, edge_index, W1, b1, W2, b2):
    """Host-side sharding/layout prep (index manipulation only)."""
    import ml_dtypes

    src = edge_index[0].astype(np.int64)
    dst = edge_index[1].astype(np.int64)
    indeg = np.bincount(dst, minlength=N)

    # degree-sorted round-robin shard assignment
    order = np.argsort(-indeg, kind="stable")      # rank -> node
    node_core = np.empty(N, np.int64)
    node_slot = np.empty(N, np.int64)
    node_core[order] = np.arange(N) % N_CORES
    node_slot[order] = np.arange(N) // N_CORES
    table_row = node_core * SHARD_PAD + node_slot   # node -> table row

    # per-core CSR of in-edges in slot order, slot0 = self loop
    # tile schedule: K per tile = max (deg+1) over the tile across all cores
    rank_deg = indeg[order]                         # degree by rank
    tile_k = []
    for t in range(TILES):
        lo = t * P * N_CORES
        tile_k.append(int(rank_deg[lo]) + 1)        # sorted desc -> first is max
    tile_off = np.zeros(TILES, np.int64)
    off = 0
    for t in range(TILES):
        tile_off[t] = off
        off += tile_k[t]
    kcols = int(off)

    # bucket edges by (core, slot)
    e_core = node_core[dst]
    e_slot = node_slot[dst]
    gidx_all = np.empty((N_CORES, P, kcols), np.int32)
    # zero rows: slot >= SHARD of own shard are zero rows in every table
    zero_row = np.arange(N_CORES) * SHARD_PAD + SHARD  # per core a junk-zero row
    for c in range(N_CORES):
        gidx_all[c, :, :] = zero_row[c]
    # order edges by (core, slot) then fill sequentially
    eo = np.lexsort((src, e_slot, e_core))
    sc, ss, ssrc = e_core[eo], e_slot[eo], src[eo]
    # position of each edge within its destination's list (after self at k=0)
    # run-length: edges sorted by (core, slot): index within group
    grp = sc * SHARD + ss
    first = np.ones(len(grp), bool)
    first[1:] = grp[1:] != grp[:-1]
    gstart = np.flatnonzero(first)
    within = np.arange(len(grp)) - np.repeat(gstart, np.diff(np.append(gstart, len(grp))))
    t_of_slot = ss // P
    p_of_slot = ss % P
    col = tile_off[t_of_slot] + 1 + within          # k = 1 + within (k=0 is self)
    gidx_all[sc, p_of_slot, col] = table_row[ssrc].astype(np.int32)
    # self loops at k = 0
    for c in range(N_CORES):
        own = np.flatnonzero(node_core == c)
        sl = node_slot[own]
        gidx_all[c, sl % P, tile_off[sl // P]] = table_row[own].astype(np.int32)

    # xT in table-row order, zero-padded junk rows, bf16.
    # dinv is pre-folded into x: ht = dinv*(x@W1) == ((dinv*x)@W1).
    dinv_arr = (1.0 / np.sqrt(1.0 + indeg)).astype(np.float32)
    xT = np.zeros((ROWS, D_IN), np.float32)
    xT[table_row] = x * dinv_arr[:, None]
    xT = np.ascontiguousarray(xT.T).astype(ml_dtypes.bfloat16)  # [128, ROWS]

    # cnt in table-row order [P, ROWS//P]: row r at (r%P, r//P)
    cnt_rows = np.zeros(ROWS, np.int32)
    cnt_rows[table_row] = indeg.astype(np.int32)
    cntg = cnt_rows.reshape(ROWS // P, P).T.copy()

    # per-core tile-order cnt + valid mask
    cnts_all = np.zeros((N_CORES, P, TILES), np.int32)
    masks_all = np.zeros((N_CORES, P, TILES), np.float32)
    for c in range(N_CORES):
        own = np.flatnonzero(node_core == c)
        sl = node_slot[own]
        cnts_all[c, sl % P, sl // P] = indeg[own].astype(np.int32)
        masks_all[c, sl % P, sl // P] = 1.0
    b1r = np.tile(b1[None, :], (P, 1)).astype(np.float32)
    b2r = np.tile(b2[None, :], (P, 1)).astype(np.float32)

    return dict(
        xT=xT, cntg=cntg, cnts_all=cnts_all, masks_all=masks_all,
        gidx_all=gidx_all, b1r=b1r, b2r=b2r,
        tile_off=tile_off, tile_k=tile_k, kcols=kcols,
        node_core=node_core, node_slot=node_slot,
    )


_CACHE = {}


def kernel(x, edge_index, W1, b1, W2, b2):
    # register the axon NTFF hook shim so bass_utils imports cleanly
    if "antenv.axon_hooks" not in sys.modules:
        m = types.ModuleType("antenv.axon_hooks")
        m._h = None
        m.set_axon_ntff_profile_hook = lambda h: setattr(m, "_h", h)
        m.get_axon_ntff_profile_hook = lambda: m._h
        sys.modules["antenv.axon_hooks"] = m

    from concourse.bass_utils import run_bass_kernel_spmd

    x = np.asarray(x, np.float32)
    edge_index = np.asarray(edge_index, np.int32)
    W1 = np.asarray(W1, np.float32)
    b1 = np.asarray(b1, np.float32)
    W2 = np.asarray(W2, np.float32)
    b2 = np.asarray(b2, np.float32)

    pr = _prep(x, edge_index, W1, b1, W2, b2)

    key = ("gcn", pr["kcols"], tuple(pr["tile_k"]))
    if key not in _CACHE:
        _CACHE[key] = _build_bass(pr["kcols"], pr["tile_off"], pr["tile_k"])
    nc = _CACHE[key]

    in_maps = []
    for c in range(N_CORES):
        in_maps.append({
            "xT": pr["xT"],
            "W1": W1, "W2": W2, "b1r": pr["b1r"], "b2r": pr["b2r"],
            "cntg": pr["cntg"],
            "cnts": pr["cnts_all"][c],
            "masks": pr["masks_all"][c],
            "gidx": pr["gidx_all"][c],
        })
    res = run_bass_kernel_spmd(nc, in_maps, core_ids=list(range(N_CORES)),
                               trace=bool(int(os.environ.get("GCN_TRACE", "0"))))
    kernel.last_exec_ns = res.exec_time_ns

    out = np.empty((N, D_OUT), np.float32)
    for c in range(N_CORES):
        own = np.flatnonzero(pr["node_core"] == c)
        out[own] = res.results[c]["out"][pr["node_slot"][own]]
    return out


if __name__ == "__main__":
    rng = np.random.default_rng(0)
    xs = rng.standard_normal((N, D_IN)).astype(np.float32)
    ei = rng.integers(0, N, (2, E)).astype(np.int32)
    w1 = rng.standard_normal((D_IN, H)).astype(np.float32) / np.sqrt(D_IN)
    w2 = rng.standard_normal((H, D_OUT)).astype(np.float32) / np.sqrt(H)
    o = kernel(xs, ei, w1, np.zeros(H, np.float32), w2, np.zeros(D_OUT, np.float32))
    print(o.shape, kernel.last_exec_ns)

